# revision 35
# baseline (speedup 1.0000x reference)
"""Causal self-attention (T=2048, B=2, d_model=1024, 16 heads) on 8 TRN2 cores.

Sharding (tensor parallel over heads + data parallel over batch):
  core (b, hg) with b in {0,1}, hg in {0..3} owns batch b and heads
  [4*hg, 4*hg+4).  Each core computes q/k/v projections for its 4 heads,
  causal flash-style attention, and its partial o_proj contribution
  out_partial = ctx_local @ Wo[:, local_dims].T.  The host sums the four
  partials per batch (the "all-reduce") and interleaves the two batches.

Per-core kernel layout (all matmul operands fp16, accumulation fp32 PSUM):
  - activations kept feature-major (d on partitions) so no transposes:
      Q.T/K.T [256, T], V token-major [T, 4, 65] with a fused ones column
      that makes the PV matmul emit the softmax denominator for free.
  - S.T = K_tile.T x Q.T per 128-wide k-tile / 512-wide q-chunk, with the
    matmul N trimmed to the causally valid span; exp (with the 1/sqrt(d)
    scale folded in) runs on the scalar engine straight out of PSUM; the
    diagonal 128x128 block is masked by a triangular multiply on DVE.
    k-tiles are processed in pairs so the PE's 64-row (QK) and 128-row
    (PV) tiling modes alternate half as often (mode switches drain).
  - softmax max-subtraction is skipped: scores are ~N(0,1) (bounded by
    construction), exp cannot overflow fp32/fp16 here.
  - inputs are host-prearranged to [ki, ko, n] so DMA rows are long
    contiguous runs; x is fully SBUF-resident, loaded chunk-major so the
    first projection is never paced by later chunks' bytes; dummy
    matmuls on a zero tile warm the PE HAM clock gate during the wait.
  - projection (A) and o_proj (C) pieces are interleaved one-by-one into
    the attention (B) instruction stream so the tensor engine always has
    filler while the scalar engine chews exp; all o_proj work is
    deferred into the exp-bound final attention chunk.
  - the final normalize runs a latency-optimized chain (ACT den copies
    first, fast reciprocal, tensor-engine K=1 broadcast) with keepalive
    matmuls holding the clock gate open; output is written fp16 and the
    host sums the four per-batch partials in float64.
"""

import numpy as np

import concourse.mybir as mybir
import concourse.tile as tile
from concourse import bacc
from concourse.bass import ds, ts
from concourse.bass_utils import run_bass_kernel_spmd

F32 = mybir.dt.float32
MM = mybir.dt.float16
AF = mybir.ActivationFunctionType

T = 2048
C = 1024
NH = 4            # heads per core
D = 64
DL = NH * D       # 256 local head dims
NCHUNK = T // 512
NKT = T // 128

_CACHE = {}


def _build():
    nc = bacc.Bacc("TRN2", target_bir_lowering=False, debug=False)

    # inputs are host-prearranged to [ki, ko, ...] so every DMA row is a
    # long contiguous run (2-4 KB descriptors instead of 0.5-1 KB)
    xT = nc.dram_tensor("xT", [128, 8, T], MM, kind="ExternalInput").ap()
    wqT = nc.dram_tensor("wqT", [128, 8, DL], MM, kind="ExternalInput").ap()
    wkT = nc.dram_tensor("wkT", [128, 8, DL], MM, kind="ExternalInput").ap()
    wvT = nc.dram_tensor("wvT", [128, 8, DL], MM, kind="ExternalInput").ap()
    woT = nc.dram_tensor("woT", [128, 2, C], MM, kind="ExternalInput").ap()
    out = nc.dram_tensor("out", [T, C], MM, kind="ExternalOutput").ap()

    with tile.TileContext(nc) as tc:
        with (
            tc.tile_pool(name="persist", bufs=1) as persist,
            tc.tile_pool(name="ptp", bufs=6) as ptp,
            tc.tile_pool(name="small", bufs=2) as small,
            tc.tile_pool(name="outp", bufs=3) as outp,
            tc.tile_pool(name="ppQK", bufs=2, space="PSUM") as ppQK,
            tc.tile_pool(name="ppST", bufs=2, space="PSUM") as ppST,
            tc.tile_pool(name="ppPV", bufs=1, space="PSUM") as ppPV,
        ):
            wq_sb = persist.tile([128, 8, DL], MM, tag="wq")
            wk_sb = persist.tile([128, 8, DL], MM, tag="wk")
            wv_sb = persist.tile([128, 8, DL], MM, tag="wv")
            wo_sb = persist.tile([128, 2, C], MM, tag="wo")
            qT_sb = persist.tile([128, 2, T], MM, tag="qT")
            kT_sb = persist.tile([128, 2, T], MM, tag="kT")
            v_sb = persist.tile([128, NKT, NH, D + 1], MM, tag="v")
            # ctx packed per head pair: partitions 0-63 head 2m, 64-127 head 2m+1
            ctx_sb = [persist.tile([128, T], MM, tag=f"ctx{m}", name=f"ctxp{m}")
                      for m in range(2)]
            warm = persist.tile([128, 512], MM, tag="warm")
            x_sb = persist.tile([128, 8, T], MM, tag="x")

            # ---- startup: first-needed DMA slices, then HAM warm-up ----
            # x loads are chunk-major so chunk 0 lands first and phase A
            # is never paced by later chunks' bytes
            nc.gpsimd.memset(warm[:], 0.0)
            nc.sync.dma_start(wq_sb[:, 0, :], wqT[:, 0, :])
            nc.sync.dma_start(x_sb[:, 0, 0:512], xT[:, 0, 0:512])
            wps = ppQK.tile([128, 512], F32, tag="qk", name="warmps")
            for _ in range(12):
                nc.tensor.matmul(wps[:], warm[:, 0:128], warm[:],
                                 start=True, stop=True)
            # guard read so the warm-up chain is never dead-code
            nc.vector.tensor_copy(warm[0:1, 0:1], wps[0:1, 0:1])
            for ko in range(1, 8):
                nc.sync.dma_start(wq_sb[:, ko, :], wqT[:, ko, :])
                nc.sync.dma_start(x_sb[:, ko, 0:512], xT[:, ko, 0:512])
            for ko in range(8):
                nc.sync.dma_start(wk_sb[:, ko, :], wkT[:, ko, :])
            for ko in range(8):
                nc.sync.dma_start(wv_sb[:, ko, :], wvT[:, ko, :])
            for nci in range(1, NCHUNK):
                nsl = ds(nci * 512, 512)
                for ko in range(8):
                    nc.sync.dma_start(x_sb[:, ko, nsl], xT[:, ko, nsl])
            ones_f32 = persist.tile([128, NKT, NH, 1], F32, tag="ones")
            nc.gpsimd.memset(ones_f32[:], 1.0)
            nc.vector.tensor_copy(v_sb[:, :, :, D:D + 1], ones_f32[:])
            # K=1 all-ones stationary for tensor-engine partition broadcast
            ones64f = persist.tile([1, 64], F32, tag="ones64f")
            nc.gpsimd.memset(ones64f[:], 1.0)
            # tri[x, y] = 1.0 if y >= x else 0.0 (diagonal-block causal mask)
            tri = persist.tile([128, 128], MM, tag="tri")
            nc.gpsimd.memset(tri[:], 1.0)
            nc.gpsimd.affine_select(
                out=tri[:], in_=tri[:], compare_op=mybir.AluOpType.is_ge,
                fill=0.0, base=0, channel_multiplier=-1, pattern=[[1, 128]])

            def make_a(nci):
                """Projection pieces for chunk nci (x is fully resident).

                PSUM evacuation goes to ScalarE for early chunks (idle
                then) and VectorE for the last (ACT is exp-bound late)."""
                nsl = ds(nci * 512, 512)

                def qk_group(w_sb, m, dst):
                    def f():
                        ps = ppQK.tile([128, 512], F32, tag="qk")
                        for ko in range(8):
                            nc.tensor.matmul(
                                ps[:], w_sb[:, ko, ts(m, 128)],
                                x_sb[:, ko, nsl],
                                start=(ko == 0), stop=(ko == 7))
                        nc.vector.tensor_copy(dst[:, m, nsl], ps[:])
                    return f

                def v_group(ki):
                    def f():
                        pv = ppQK.tile([128, 512], F32, tag="qk")
                        for ko in range(8):
                            nc.tensor.matmul(
                                pv[:, 0:DL],
                                x_sb[:, ko, ds(nci * 512 + ki * 128, 128)],
                                wv_sb[:, ko, :],
                                start=(ko == 0), stop=(ko == 7))
                        nc.vector.tensor_copy(
                            v_sb[:, nci * 4 + ki, :, 0:D],
                            pv[:, 0:DL].rearrange("p (h d) -> p h d", d=D))
                    return f

                return [qk_group(wq_sb, 0, qT_sb), qk_group(wk_sb, 0, kT_sb),
                        qk_group(wq_sb, 1, qT_sb), qk_group(wk_sb, 1, kT_sb),
                        v_group(0), v_group(1), v_group(2), v_group(3)]

            def make_b_steps(qc):
                """Attention steps for chunk qc: per-m lists of closures.

                ki tiles are processed in PAIRS so the tensor engine's
                64-row (QK) and 128-row (PV) tiling modes alternate half as
                often (each mode switch drains the PE pipeline)."""
                qoff = qc * 512
                nk = 4 * qc + 4
                per_m = []
                for m in range(2):
                    steps = []
                    pvps = ppPV.tile([D + 1, 2, 512], F32, tag="pv",
                                     name=f"pv{qc}_{m}")
                    state = {}

                    def do_qk(ki, m=m):
                        voff = max(0, ki * 128 - qoff)
                        span = 512 - voff
                        st = ppST.tile([128, 2, 512], F32, tag="st")
                        for sub in range(2):
                            po = 64 * sub
                            nc.tensor.matmul(
                                st[:, sub, voff:512],
                                kT_sb[po:po + 64, m, ds(ki * 128, 128)],
                                qT_sb[po:po + 64, m, ds(qoff + voff, span)],
                                start=True, stop=True)
                        return (st, voff, ki)

                    def do_exp(stv, m=m):
                        st, voff, ki = stv
                        pt = ptp.tile([128, 2, 512], MM, tag="pt")
                        nc.scalar.activation(
                            pt[:, :, voff:512], st[:, :, voff:512],
                            AF.Exp, scale=0.125)
                        if ki * 128 >= qoff:
                            nc.vector.tensor_mul(
                                pt[:, :, voff:voff + 128],
                                pt[:, :, voff:voff + 128],
                                tri[:, None, :].to_broadcast([128, 2, 128]))
                        return (pt, voff, ki)

                    def do_pv(ptv, last, m=m, pvps=pvps):
                        ppt, pvoff, pki = ptv
                        for sub in range(2):
                            nc.tensor.matmul(
                                pvps[:, sub, pvoff:512],
                                v_sb[:, pki, 2 * m + sub, :],
                                ppt[:, sub, pvoff:512],
                                start=(pki == 0), stop=last)

                    def pair_step(state=state, do_qk=do_qk, do_exp=do_exp,
                                  do_pv=do_pv):
                        p = state.setdefault("p", 0)
                        state["p"] = p + 1
                        new = [do_qk(2 * p), do_qk(2 * p + 1)]
                        pts = [do_exp(sv) for sv in new]
                        for ptv in state.get("pend", []):
                            do_pv(ptv, False)
                        state["pend"] = pts

                    def flush(state=state, nk=nk, do_pv=do_pv):
                        for ptv in state["pend"]:
                            do_pv(ptv, ptv[2] == nk - 1)

                    def norm(sub, m=m, pvps=pvps, qc=qc):
                        def f():
                            den = small.tile([1, 512], F32, tag="den")
                            sbc = small.tile([D, 512], F32, tag="sbv")
                            nc.vector.tensor_copy(
                                den[:], pvps[D:D + 1, sub, :])
                            nc.vector.tensor_copy(
                                sbc[:], pvps[0:D, sub, :])
                            inv = small.tile([1, 512], F32, tag="inv")
                            nc.vector.reciprocal_approx_fast(
                                out=inv[:], in_=den[:])
                            rec = small.tile([D, 512], F32, tag="rec")
                            nc.gpsimd.partition_broadcast(rec[:], inv[:])
                            nc.vector.tensor_mul(
                                ctx_sb[m][64 * sub:64 * sub + 64,
                                          ds(qc * 512, 512)],
                                sbc[:], rec[:])
                        return f

                    def norm_tail(m=m, pvps=pvps, qc=qc):
                        # latency-optimized final normalize: den copies lead
                        # on ACT, broadcasts run as K=1 matmuls on the (idle)
                        # tensor engine, and dummy keepalive matmuls hold the
                        # HAM clock gate open so the o_proj tail runs warm
                        dens, sbcs, invs = [], [], []
                        for sub in range(2):
                            den = small.tile([1, 512], F32, tag="den")
                            nc.scalar.copy(den[:], pvps[D:D + 1, sub, :])
                            dens.append(den)
                        ka = ppST.tile([128, 2, 512], F32, tag="st",
                                       name="keepalive")
                        for _ in range(8):
                            nc.tensor.matmul(ka[:, 0, :], warm[:, 0:128],
                                             warm[:], start=True, stop=True)
                        nc.vector.tensor_copy(warm[0:1, 1:2], ka[0:1, 0, 0:1])
                        for sub in range(2):
                            inv = small.tile([1, 512], F32, tag="inv")
                            nc.vector.reciprocal_approx_fast(
                                out=inv[:], in_=dens[sub][:])
                            invs.append(inv)
                        for sub in range(2):
                            sbc = small.tile([D, 512], F32, tag="sbv")
                            nc.scalar.copy(sbc[:], pvps[0:D, sub, :])
                            sbcs.append(sbc)
                        recs = []
                        for sub in range(2):
                            rec = ppQK.tile([D, 512], F32, tag="qk",
                                            name=f"recps{sub}")
                            nc.tensor.matmul(rec[:], ones64f[:], invs[sub][:],
                                             start=True, stop=True)
                            recs.append(rec)
                        for sub in range(2):
                            nc.vector.tensor_mul(
                                ctx_sb[m][64 * sub:64 * sub + 64,
                                          ds(qc * 512, 512)],
                                sbcs[sub][:], recs[sub][:])

                    steps.extend([pair_step] * (nk // 2))
                    steps.append(flush)
                    if qc == NCHUNK - 1 and m == 1:
                        steps.append(norm_tail)
                    else:
                        steps.append(norm(0))
                        steps.append(norm(1))
                    per_m.append(steps)
                return per_m

            def make_c(qc):
                """o_proj pieces for the 4 token tiles of chunk qc."""
                pieces = []
                for nt in range(4 * qc, 4 * qc + 4):
                    def f(nt=nt, qc=qc):
                        tsl = ds(nt * 128, 128)
                        osb = outp.tile([128, 1024], MM, tag="osb")
                        for cc in range(2):
                            csl = ds(cc * 512, 512)
                            ops = ppQK.tile([128, 512], F32, tag="qk")
                            for m in range(2):
                                nc.tensor.matmul(
                                    ops[:], ctx_sb[m][:, tsl],
                                    wo_sb[:, m, csl],
                                    start=(m == 0), stop=(m == 1))
                            if qc == NCHUNK - 1:
                                nc.scalar.copy(osb[:, csl], ops[:])
                            else:
                                nc.vector.tensor_copy(osb[:, csl], ops[:])
                            nc.sync.dma_start(out[tsl, csl], osb[:, csl])
                    pieces.append(f)
                return pieces

            # ---- main interleaved emission ----
            def emit_interleaved(steps, fill, lead=False):
                # emission order defines Tile RAW deps: a fill piece whose
                # output a step consumes MUST be emitted before that step
                # (lead=True emits fill i before step i, 1:1 from the start)
                nf, ns = len(fill), len(steps)
                fi = 0
                for si, step in enumerate(steps):
                    if lead:
                        while fi < nf and fi <= si:
                            fill[fi]()
                            fi += 1
                    step()
                    if not lead:
                        while fi < nf and fi * ns <= (si + 1) * nf:
                            fill[fi]()
                            fi += 1
                while fi < nf:
                    fill[fi]()
                    fi += 1

            a0 = make_a(0)
            b0 = make_b_steps(0)
            a0[0]()           # Q m0
            a0[1]()           # K m0
            nc.sync.dma_start(wo_sb[:], woT)   # o_proj weights: needed late
            # V fills lead 2 per pair-step: PV(pair p) consumes v tiles
            # 2p and 2p+1 one pair-step later
            a0[4]()
            a0[5]()
            b0[0][0]()
            a0[6]()
            a0[7]()
            for s in b0[0][1:]:
                s()
            a0[2]()           # Q m1
            a0[3]()           # K m1
            a1 = make_a(1)
            emit_interleaved(b0[1], a1[0:4])
            c0 = make_c(0)
            b1 = make_b_steps(1)
            a2 = make_a(2)
            emit_interleaved(b1[0] + b1[1], a1[4:8] + a2)
            c1 = make_c(1)
            b2 = make_b_steps(2)
            a3 = make_a(3)
            emit_interleaved(b2[0] + b2[1], a3)
            c2 = make_c(2)
            b3 = make_b_steps(3)
            # o_proj of chunks 0-2 fills the ACT-bound last attention chunk
            emit_interleaved(b3[0] + b3[1], c0 + c1 + c2)
            for p in make_c(3):
                p()

    nc.compile()
    return nc


def _ko_major(a):
    # [C, n] -> [ki, ko, n] materialized contiguous (C = ko*128 + ki)
    return np.ascontiguousarray(
        a.reshape(8, 128, a.shape[1]).transpose(1, 0, 2))


def _host_prep(x, Wq, Wk, Wv, Wo, b, hg):
    sl = slice(hg * DL, (hg + 1) * DL)
    xT = _ko_major(x[:, b, :].T)
    wqT = _ko_major(Wq[sl, :].T)
    wkT = _ko_major(Wk[sl, :].T)
    wvT = _ko_major(Wv[sl, :].T)
    # woT[p, pair, c] = Wo[c, hg*256 + (2*pair + p//64)*64 + p%64]
    w = Wo[:, sl].T.reshape(2, 2, 64, C)
    woT = np.ascontiguousarray(w.transpose(1, 2, 0, 3).reshape(128, 2, C))
    d = {"xT": xT, "wqT": wqT, "wkT": wkT, "wvT": wvT, "woT": woT}
    return {k: v.astype(np.float16) for k, v in d.items()}


def _run(x, Wq, Wk, Wv, Wo, trace=False):
    if "nc" not in _CACHE:
        _CACHE["nc"] = _build()
    nc = _CACHE["nc"]
    in_maps = [_host_prep(x, Wq, Wk, Wv, Wo, b, hg)
               for b in range(2) for hg in range(4)]
    res = run_bass_kernel_spmd(nc, in_maps, list(range(8)), trace=trace)
    out = np.empty((T, 2, C), np.float32)
    for b in range(2):
        acc = res.results[b * 4 + 0]["out"].astype(np.float64)
        for hg in range(1, 4):
            acc += res.results[b * 4 + hg]["out"]
        out[:, b, :] = acc.astype(np.float32)
    return out, res


def kernel(x, Wq, Wk, Wv, Wo):
    x = np.ascontiguousarray(np.asarray(x, dtype=np.float32))
    Wq = np.asarray(Wq, dtype=np.float32)
    Wk = np.asarray(Wk, dtype=np.float32)
    Wv = np.asarray(Wv, dtype=np.float32)
    Wo = np.asarray(Wo, dtype=np.float32)
    out, _ = _run(x, Wq, Wk, Wv, Wo)
    return out


# revision 36
# speedup vs baseline: 1.0312x; 1.0312x over previous
"""Causal self-attention (T=2048, B=2, d_model=1024, 16 heads) on 8 TRN2 cores.

Sharding (tensor parallel over heads + data parallel over batch):
  core (b, hg) with b in {0,1}, hg in {0..3} owns batch b and heads
  [4*hg, 4*hg+4).  Each core computes q/k/v projections for its 4 heads,
  causal flash-style attention, and its partial o_proj contribution
  out_partial = ctx_local @ Wo[:, local_dims].T.  The host sums the four
  partials per batch (the "all-reduce") and interleaves the two batches.

Per-core kernel layout (all matmul operands fp16, accumulation fp32 PSUM):
  - activations kept feature-major (d on partitions) so no transposes:
      Q.T/K.T [256, T], V token-major [T, 4, 65] with a fused ones column
      that makes the PV matmul emit the softmax denominator for free.
  - S.T = K_tile.T x Q.T per 128-wide k-tile / 512-wide q-chunk, with the
    matmul N trimmed to the causally valid span; exp (with the 1/sqrt(d)
    scale folded in) runs on the scalar engine straight out of PSUM; the
    diagonal 128x128 block is masked by a triangular multiply on DVE.
    k-tiles are processed in pairs so the PE's 64-row (QK) and 128-row
    (PV) tiling modes alternate half as often (mode switches drain).
  - softmax max-subtraction is skipped: scores are ~N(0,1) (bounded by
    construction), exp cannot overflow fp32/fp16 here.
  - inputs are host-prearranged to [ki, ko, n] so DMA rows are long
    contiguous runs; x is fully SBUF-resident, loaded chunk-major so the
    first projection is never paced by later chunks' bytes; dummy
    matmuls on a zero tile warm the PE HAM clock gate during the wait.
  - projection (A) and o_proj (C) pieces are interleaved one-by-one into
    the attention (B) instruction stream so the tensor engine always has
    filler while the scalar engine chews exp; all o_proj work is
    deferred into the exp-bound final attention chunk.
  - the final normalize runs a latency-optimized chain (ACT den copies
    first, fast reciprocal, tensor-engine K=1 broadcast) with keepalive
    matmuls holding the clock gate open; output is written fp16 and the
    host sums the four per-batch partials in float64.
"""

import numpy as np

import concourse.mybir as mybir
import concourse.tile as tile
from concourse import bacc
from concourse.bass import ds, ts
from concourse.bass_utils import run_bass_kernel_spmd

F32 = mybir.dt.float32
MM = mybir.dt.float16
AF = mybir.ActivationFunctionType

T = 2048
C = 1024
NH = 4            # heads per core
D = 64
DL = NH * D       # 256 local head dims
NCHUNK = T // 512
NKT = T // 128

_CACHE = {}


def _build():
    nc = bacc.Bacc("TRN2", target_bir_lowering=False, debug=False)

    # inputs are host-prearranged to [ki, ko, ...] so every DMA row is a
    # long contiguous run (2-4 KB descriptors instead of 0.5-1 KB)
    xT = nc.dram_tensor("xT", [128, 8, T], MM, kind="ExternalInput").ap()
    wqT = nc.dram_tensor("wqT", [128, 8, DL], MM, kind="ExternalInput").ap()
    wkT = nc.dram_tensor("wkT", [128, 8, DL], MM, kind="ExternalInput").ap()
    wvT = nc.dram_tensor("wvT", [128, 8, DL], MM, kind="ExternalInput").ap()
    woT = nc.dram_tensor("woT", [128, 2, C], MM, kind="ExternalInput").ap()
    out = nc.dram_tensor("out", [T, C], MM, kind="ExternalOutput").ap()

    with tile.TileContext(nc) as tc:
        with (
            tc.tile_pool(name="persist", bufs=1) as persist,
            tc.tile_pool(name="ptp", bufs=6) as ptp,
            tc.tile_pool(name="small", bufs=2) as small,
            tc.tile_pool(name="outp", bufs=3) as outp,
            tc.tile_pool(name="ppQK", bufs=2, space="PSUM") as ppQK,
            tc.tile_pool(name="ppST", bufs=2, space="PSUM") as ppST,
            tc.tile_pool(name="ppPV", bufs=1, space="PSUM") as ppPV,
        ):
            wq_sb = persist.tile([128, 8, DL], MM, tag="wq")
            wk_sb = persist.tile([128, 8, DL], MM, tag="wk")
            wv_sb = persist.tile([128, 8, DL], MM, tag="wv")
            wo_sb = persist.tile([128, 2, C], MM, tag="wo")
            qT_sb = persist.tile([128, 2, T], MM, tag="qT")
            kT_sb = persist.tile([128, 2, T], MM, tag="kT")
            v_sb = persist.tile([128, NKT, NH, D + 1], MM, tag="v")
            # ctx packed per head pair: partitions 0-63 head 2m, 64-127 head 2m+1
            ctx_sb = [persist.tile([128, T], MM, tag=f"ctx{m}", name=f"ctxp{m}")
                      for m in range(2)]
            warm = persist.tile([128, 512], MM, tag="warm")
            x_sb = persist.tile([128, 8, T], MM, tag="x")

            # ---- startup: first-needed DMA slices, then HAM warm-up ----
            # x loads are chunk-major so chunk 0 lands first and phase A
            # is never paced by later chunks' bytes
            nc.gpsimd.memset(warm[:], 0.0)
            nc.sync.dma_start(wq_sb[:, 0, :], wqT[:, 0, :])
            nc.sync.dma_start(x_sb[:, 0, 0:512], xT[:, 0, 0:512])
            wps = ppQK.tile([128, 512], F32, tag="qk", name="warmps")
            for _ in range(12):
                nc.tensor.matmul(wps[:], warm[:, 0:128], warm[:],
                                 start=True, stop=True)
            # guard read so the warm-up chain is never dead-code
            nc.vector.tensor_copy(warm[0:1, 0:1], wps[0:1, 0:1])
            for ko in range(1, 8):
                nc.sync.dma_start(wq_sb[:, ko, :], wqT[:, ko, :])
                nc.sync.dma_start(x_sb[:, ko, 0:512], xT[:, ko, 0:512])
            for ko in range(8):
                nc.sync.dma_start(wk_sb[:, ko, :], wkT[:, ko, :])
            for ko in range(8):
                nc.sync.dma_start(wv_sb[:, ko, :], wvT[:, ko, :])
            for nci in range(1, NCHUNK):
                nsl = ds(nci * 512, 512)
                for ko in range(8):
                    nc.sync.dma_start(x_sb[:, ko, nsl], xT[:, ko, nsl])
            ones_f32 = persist.tile([128, NKT, NH, 1], F32, tag="ones")
            nc.gpsimd.memset(ones_f32[:], 1.0)
            nc.vector.tensor_copy(v_sb[:, :, :, D:D + 1], ones_f32[:])
            # K=1 all-ones stationary for tensor-engine partition broadcast
            ones64f = persist.tile([1, 64], F32, tag="ones64f")
            nc.gpsimd.memset(ones64f[:], 1.0)
            # tri[x, y] = 1.0 if y >= x else 0.0 (diagonal-block causal mask)
            tri = persist.tile([128, 128], MM, tag="tri")
            nc.gpsimd.memset(tri[:], 1.0)
            nc.gpsimd.affine_select(
                out=tri[:], in_=tri[:], compare_op=mybir.AluOpType.is_ge,
                fill=0.0, base=0, channel_multiplier=-1, pattern=[[1, 128]])

            def make_a(nci):
                """Projection pieces for chunk nci (x is fully resident).

                PSUM evacuation goes to ScalarE for early chunks (idle
                then) and VectorE for the last (ACT is exp-bound late)."""
                nsl = ds(nci * 512, 512)

                def qk_group(w_sb, m, dst):
                    def f():
                        ps = ppQK.tile([128, 512], F32, tag="qk")
                        for ko in range(8):
                            nc.tensor.matmul(
                                ps[:], w_sb[:, ko, ts(m, 128)],
                                x_sb[:, ko, nsl],
                                start=(ko == 0), stop=(ko == 7))
                        nc.vector.tensor_copy(dst[:, m, nsl], ps[:])
                    return f

                def v_group(ki):
                    def f():
                        pv = ppQK.tile([128, 512], F32, tag="qk")
                        for ko in range(8):
                            nc.tensor.matmul(
                                pv[:, 0:DL],
                                x_sb[:, ko, ds(nci * 512 + ki * 128, 128)],
                                wv_sb[:, ko, :],
                                start=(ko == 0), stop=(ko == 7))
                        nc.vector.tensor_copy(
                            v_sb[:, nci * 4 + ki, :, 0:D],
                            pv[:, 0:DL].rearrange("p (h d) -> p h d", d=D))
                    return f

                return [qk_group(wq_sb, 0, qT_sb), qk_group(wk_sb, 0, kT_sb),
                        qk_group(wq_sb, 1, qT_sb), qk_group(wk_sb, 1, kT_sb),
                        v_group(0), v_group(1), v_group(2), v_group(3)]

            def make_b_steps(qc):
                """Attention steps for chunk qc: per-m lists of closures.

                ki tiles are processed in PAIRS so the tensor engine's
                64-row (QK) and 128-row (PV) tiling modes alternate half as
                often (each mode switch drains the PE pipeline)."""
                qoff = qc * 512
                nk = 4 * qc + 4
                per_m = []
                for m in range(2):
                    steps = []
                    pvps = ppPV.tile([D + 1, 2, 512], F32, tag="pv",
                                     name=f"pv{qc}_{m}")
                    state = {}

                    def do_qk(ki, m=m):
                        voff = max(0, ki * 128 - qoff)
                        span = 512 - voff
                        st = ppST.tile([128, 2, 512], F32, tag="st")
                        for sub in range(2):
                            po = 64 * sub
                            nc.tensor.matmul(
                                st[:, sub, voff:512],
                                kT_sb[po:po + 64, m, ds(ki * 128, 128)],
                                qT_sb[po:po + 64, m, ds(qoff + voff, span)],
                                start=True, stop=True)
                        return (st, voff, ki)

                    def do_exp(stv, m=m):
                        st, voff, ki = stv
                        pt = ptp.tile([128, 2, 512], MM, tag="pt")
                        nc.scalar.activation(
                            pt[:, :, voff:512], st[:, :, voff:512],
                            AF.Exp, scale=0.125)
                        if ki * 128 >= qoff:
                            nc.vector.tensor_mul(
                                pt[:, :, voff:voff + 128],
                                pt[:, :, voff:voff + 128],
                                tri[:, None, :].to_broadcast([128, 2, 128]))
                        return (pt, voff, ki)

                    def do_pv(ptv, last, m=m, pvps=pvps):
                        ppt, pvoff, pki = ptv
                        for sub in range(2):
                            nc.tensor.matmul(
                                pvps[:, sub, pvoff:512],
                                v_sb[:, pki, 2 * m + sub, :],
                                ppt[:, sub, pvoff:512],
                                start=(pki == 0), stop=last)

                    def pair_step(state=state, do_qk=do_qk, do_exp=do_exp,
                                  do_pv=do_pv):
                        p = state.setdefault("p", 0)
                        state["p"] = p + 1
                        new = [do_qk(2 * p), do_qk(2 * p + 1)]
                        pts = [do_exp(sv) for sv in new]
                        for ptv in state.get("pend", []):
                            do_pv(ptv, False)
                        state["pend"] = pts

                    def flush(state=state, nk=nk, do_pv=do_pv):
                        for ptv in state["pend"]:
                            do_pv(ptv, ptv[2] == nk - 1)

                    def norm(sub, m=m, pvps=pvps, qc=qc):
                        def f():
                            den = small.tile([1, 512], F32, tag="den")
                            sbc = small.tile([D, 512], F32, tag="sbv")
                            nc.vector.tensor_copy(
                                den[:], pvps[D:D + 1, sub, :])
                            nc.vector.tensor_copy(
                                sbc[:], pvps[0:D, sub, :])
                            inv = small.tile([1, 512], F32, tag="inv")
                            nc.vector.reciprocal_approx_fast(
                                out=inv[:], in_=den[:])
                            rec = small.tile([D, 512], F32, tag="rec")
                            nc.gpsimd.partition_broadcast(rec[:], inv[:])
                            nc.vector.tensor_mul(
                                ctx_sb[m][64 * sub:64 * sub + 64,
                                          ds(qc * 512, 512)],
                                sbc[:], rec[:])
                        return f

                    def norm_tail(m=m, pvps=pvps, qc=qc):
                        # latency-optimized final normalize: den copies lead
                        # on ACT, broadcasts run as K=1 matmuls on the (idle)
                        # tensor engine, and dummy keepalive matmuls hold the
                        # HAM clock gate open so the o_proj tail runs warm
                        dens, sbcs, invs = [], [], []
                        for sub in range(2):
                            den = small.tile([1, 512], F32, tag="den")
                            nc.scalar.copy(den[:], pvps[D:D + 1, sub, :])
                            dens.append(den)
                        ka = ppST.tile([128, 2, 512], F32, tag="st",
                                       name="keepalive")
                        for _ in range(8):
                            nc.tensor.matmul(ka[:, 0, :], warm[:, 0:128],
                                             warm[:], start=True, stop=True)
                        nc.vector.tensor_copy(warm[0:1, 1:2], ka[0:1, 0, 0:1])
                        for sub in range(2):
                            inv = small.tile([1, 512], F32, tag="inv")
                            nc.vector.reciprocal_approx_fast(
                                out=inv[:], in_=dens[sub][:])
                            invs.append(inv)
                        for sub in range(2):
                            sbc = small.tile([D, 512], F32, tag="sbv")
                            nc.scalar.copy(sbc[:], pvps[0:D, sub, :])
                            sbcs.append(sbc)
                        recs = []
                        for sub in range(2):
                            rec = ppQK.tile([D, 512], F32, tag="qk",
                                            name=f"recps{sub}")
                            nc.tensor.matmul(rec[:], ones64f[:], invs[sub][:],
                                             start=True, stop=True)
                            recs.append(rec)
                        for sub in range(2):
                            nc.vector.tensor_mul(
                                ctx_sb[m][64 * sub:64 * sub + 64,
                                          ds(qc * 512, 512)],
                                sbcs[sub][:], recs[sub][:])

                    steps.extend([pair_step] * (nk // 2))
                    steps.append(flush)
                    if qc == NCHUNK - 1 and m == 1:
                        steps.append(norm_tail)
                    else:
                        steps.append(norm(0))
                        steps.append(norm(1))
                    per_m.append(steps)
                return per_m

            def make_c(qc):
                """o_proj pieces for the 4 token tiles of chunk qc."""
                pieces = []
                for nt in range(4 * qc, 4 * qc + 4):
                    def f(nt=nt, qc=qc):
                        tsl = ds(nt * 128, 128)
                        osb = outp.tile([128, 1024], MM, tag="osb")
                        for cc in range(2):
                            csl = ds(cc * 512, 512)
                            ops = ppQK.tile([128, 512], F32, tag="qk")
                            for m in range(2):
                                nc.tensor.matmul(
                                    ops[:], ctx_sb[m][:, tsl],
                                    wo_sb[:, m, csl],
                                    start=(m == 0), stop=(m == 1))
                            if qc == NCHUNK - 1:
                                nc.scalar.copy(osb[:, csl], ops[:])
                            else:
                                nc.vector.tensor_copy(osb[:, csl], ops[:])
                            nc.sync.dma_start(out[tsl, csl], osb[:, csl])
                    pieces.append(f)
                return pieces

            # ---- main interleaved emission ----
            def emit_interleaved(steps, fill, lead=False):
                # emission order defines Tile RAW deps: a fill piece whose
                # output a step consumes MUST be emitted before that step
                # (lead=True emits fill i before step i, 1:1 from the start)
                nf, ns = len(fill), len(steps)
                fi = 0
                for si, step in enumerate(steps):
                    if lead:
                        while fi < nf and fi <= si:
                            fill[fi]()
                            fi += 1
                    step()
                    if not lead:
                        while fi < nf and fi * ns <= (si + 1) * nf:
                            fill[fi]()
                            fi += 1
                while fi < nf:
                    fill[fi]()
                    fi += 1

            a0 = make_a(0)
            b0 = make_b_steps(0)
            a0[0]()           # Q m0
            a0[1]()           # K m0
            nc.sync.dma_start(wo_sb[:], woT)   # o_proj weights: needed late
            # V fills lead 2 per pair-step: PV(pair p) consumes v tiles
            # 2p and 2p+1 one pair-step later
            a0[4]()
            a0[5]()
            b0[0][0]()
            a0[6]()
            a0[7]()
            for s in b0[0][1:]:
                s()
            a0[2]()           # Q m1
            a0[3]()           # K m1
            a1 = make_a(1)
            emit_interleaved(b0[1], a1[0:4])
            c0 = make_c(0)
            b1 = make_b_steps(1)
            a2 = make_a(2)
            emit_interleaved(b1[0] + b1[1], a1[4:8] + a2)
            c1 = make_c(1)
            b2 = make_b_steps(2)
            a3 = make_a(3)
            # C(0) joins B(2): spreads o_proj DVE casts out of the
            # DVE-tight final chunk window
            emit_interleaved(b2[0] + b2[1], a3 + c0)
            c2 = make_c(2)
            b3 = make_b_steps(3)
            # o_proj of chunks 1-2 fills the ACT-bound last attention chunk
            emit_interleaved(b3[0] + b3[1], c1 + c2)
            for p in make_c(3):
                p()

    nc.compile()
    return nc


def _ko_major(a):
    # [C, n] -> [ki, ko, n] materialized contiguous (C = ko*128 + ki)
    return np.ascontiguousarray(
        a.reshape(8, 128, a.shape[1]).transpose(1, 0, 2))


def _host_prep(x, Wq, Wk, Wv, Wo, b, hg):
    sl = slice(hg * DL, (hg + 1) * DL)
    xT = _ko_major(x[:, b, :].T)
    wqT = _ko_major(Wq[sl, :].T)
    wkT = _ko_major(Wk[sl, :].T)
    wvT = _ko_major(Wv[sl, :].T)
    # woT[p, pair, c] = Wo[c, hg*256 + (2*pair + p//64)*64 + p%64]
    w = Wo[:, sl].T.reshape(2, 2, 64, C)
    woT = np.ascontiguousarray(w.transpose(1, 2, 0, 3).reshape(128, 2, C))
    d = {"xT": xT, "wqT": wqT, "wkT": wkT, "wvT": wvT, "woT": woT}
    return {k: v.astype(np.float16) for k, v in d.items()}


def _run(x, Wq, Wk, Wv, Wo, trace=False):
    if "nc" not in _CACHE:
        _CACHE["nc"] = _build()
    nc = _CACHE["nc"]
    in_maps = [_host_prep(x, Wq, Wk, Wv, Wo, b, hg)
               for b in range(2) for hg in range(4)]
    res = run_bass_kernel_spmd(nc, in_maps, list(range(8)), trace=trace)
    out = np.empty((T, 2, C), np.float32)
    for b in range(2):
        acc = res.results[b * 4 + 0]["out"].astype(np.float64)
        for hg in range(1, 4):
            acc += res.results[b * 4 + hg]["out"]
        out[:, b, :] = acc.astype(np.float32)
    return out, res


def kernel(x, Wq, Wk, Wv, Wo):
    x = np.ascontiguousarray(np.asarray(x, dtype=np.float32))
    Wq = np.asarray(Wq, dtype=np.float32)
    Wk = np.asarray(Wk, dtype=np.float32)
    Wv = np.asarray(Wv, dtype=np.float32)
    Wo = np.asarray(Wo, dtype=np.float32)
    out, _ = _run(x, Wq, Wk, Wv, Wo)
    return out
